# revision 4
# baseline (speedup 1.0000x reference)
"""Trainium2 Bass kernel for the CAM (channel attention module) problem.

Computation (per batch b):
    A = inputs[b] reshaped [N=4096, C=512]
    G = A^T A                       (channel Gram matrix, [C, C])
    attn = softmax(G, axis=-1)
    out[b] = gamma * (A @ attn^T) + A

Distribution: pure data-parallel over the batch dim: 16 batches over 8
NeuronCores = 2 batches/core. No collectives.

Per-core dataflow (per batch, restructured for cross-batch pipelining):
  - Input is loaded via SWDGE cast-DMA (fp32 HBM -> bf16 SBUF), in a
    jt-major layout [p, jt, nt, c].  No fp32 copy of A is kept on chip;
    the residual add reads the bf16 copy (error ~2^-9 rel, far inside
    the 2e-2 gate).  This halves SBUF footprint so both batches'
    A / A^T live simultaneously -> full cross-batch overlap.
  - Gram matmul (bf16, upper-triangle blocks only) accumulates in 4
    PSUM banks; right after the last accumulation the banks are copied
    to SBUF (Gsb) so the next batch's Gram can reuse them immediately.
  - Lower-triangle blocks reconstructed by PE transposes of Gsb.
  - Softmax: row max (DVE), S = exp(G - m) on ACT with accum_out row
    sums, 1/s via DVE reciprocal; -m and 1/s are transposed to row
    vectors (PE) and broadcast across partitions with rank-1 matmuls.
  - T_w[j, i] = exp(G[j, i] - m_i) * w_i via DVE add + ACT exp + DVE mul.
  - A^T tiles for the second matmul come from the DMA xbar transpose
    (SBUF->SBUF, bf16, contiguous source).
  - Second matmul psum = A @ T_w accumulated over 4 channel blocks.
  - Residual out = psum * gamma + A_bf16 on DVE, stored in 1MB chunks.
"""

import sys

if "/opt/trn_rl_repo" not in sys.path:
    sys.path.insert(0, "/opt/trn_rl_repo")

import numpy as np

B, H, W, C = 16, 64, 64, 512
N = H * W                 # 4096
NCORES = 8
BPC = B // NCORES         # batches per core = 2
P = 128                   # partitions
NT = N // P               # 32 n-tiles
CT = C // P               # 4 channel tiles

# load chunks (in n-tiles): first two smaller so Gram starts sooner
LOAD_CHUNKS = [4, 4, 8, 8, 8]
OG = 4                    # n-tiles per output store group

_BUILD_CACHE = {}


def _ml_bf16():
    import ml_dtypes

    return np.dtype(ml_dtypes.bfloat16)


def build_bass(gamma_val: float):
    import concourse.bass as bass
    import concourse.bacc as bacc
    import concourse.tile as tile
    from concourse import mybir
    from contextlib import ExitStack

    f32 = mybir.dt.float32
    bf16 = mybir.dt.bfloat16
    Exp = mybir.ActivationFunctionType.Exp
    Alu = mybir.AluOpType
    AX = mybir.AxisListType

    nc = bacc.Bacc("TRN2", target_bir_lowering=False)
    x = nc.dram_tensor("x", [BPC, N, C], f32, kind="ExternalInput")
    ident = nc.dram_tensor("ident", [P, P], f32, kind="ExternalInput")
    ones_f = nc.dram_tensor("ones_f", [1, P], f32, kind="ExternalInput")
    ones_h = nc.dram_tensor("ones_h", [1, P], bf16, kind="ExternalInput")
    y = nc.dram_tensor("y", [BPC, N, C], f32, kind="ExternalOutput")

    with tile.TileContext(nc) as tc, ExitStack() as ctx:
        singles = ctx.enter_context(tc.tile_pool(name="singles", bufs=1))
        pAbf = ctx.enter_context(tc.tile_pool(name="pAbf", bufs=2))
        pAT = ctx.enter_context(tc.tile_pool(name="pAT", bufs=2))
        pGsb = ctx.enter_context(tc.tile_pool(name="pGsb", bufs=2))
        pTw = ctx.enter_context(tc.tile_pool(name="pTw", bufs=2))
        pS = ctx.enter_context(tc.tile_pool(name="pS", bufs=2))
        pStat = ctx.enter_context(tc.tile_pool(name="pStat", bufs=10))
        pRow = ctx.enter_context(tc.tile_pool(name="pRow", bufs=4))
        pTmp = ctx.enter_context(tc.tile_pool(name="pTmp", bufs=2))
        pOut = ctx.enter_context(tc.tile_pool(name="pOut", bufs=2))
        pG = ctx.enter_context(tc.tile_pool(name="pG", bufs=4, space="PSUM"))
        pPo = ctx.enter_context(tc.tile_pool(name="pPo", bufs=2, space="PSUM"))
        pPv = ctx.enter_context(tc.tile_pool(name="pPv", bufs=1, space="PSUM"))
        pTri = ctx.enter_context(tc.tile_pool(name="pTri", bufs=1, space="PSUM"))

        sb_ident = singles.tile([P, P], f32)
        nc.gpsimd.dma_start(out=sb_ident, in_=ident[:, :])
        sb_ones_f = singles.tile([1, P], f32)
        nc.gpsimd.dma_start(out=sb_ones_f, in_=ones_f[:, :])
        sb_ones_h = singles.tile([1, P], bf16)
        nc.gpsimd.dma_start(out=sb_ones_h, in_=ones_h[:, :])

        # ---------------- per-batch state ----------------
        Abf = [None, None]
        AT = [None, None]
        Gsb = [None, None]
        Tw = [None, None]

        def emit_loads(b):
            # Abf[b][p, jt, k, c] = bf16(x[b, k*128 + p, jt*128 + c])
            A = pAbf.tile([P, CT, NT, P], bf16, name=f"Abf_b{b}", tag="Abf")
            Abf[b] = A
            s = 0
            for kk in LOAD_CHUNKS:
                for jt in range(CT):
                    src = x[b, s * P:(s + kk) * P, jt * P:(jt + 1) * P].rearrange(
                        "(k p) c -> p k c", p=P
                    )
                    nc.gpsimd.dma_start(out=A[:, jt, s:s + kk, :], in_=src)
                s += kk

        def emit_gram(b, nt0, nt1, G):
            A = Abf[b]
            for nt in range(nt0, nt1):
                for ci in range(CT):
                    nc.tensor.matmul(
                        G[ci][:, ci * P:],
                        lhsT=A[:, ci, nt, :],
                        rhs=A[:, ci:, nt, :],
                        start=(nt == 0),
                        stop=(nt == NT - 1),
                    )

        def emit_softmax(b, G):
            # copy upper-tri G rows out of PSUM (frees banks for next batch)
            Gs = pGsb.tile([P, CT, C], f32, name=f"Gsb_b{b}", tag="Gsb")
            Gsb[b] = Gs
            for ci in range(CT):
                nc.scalar.copy(out=Gs[:, ci, ci * P:], in_=G[ci][:, ci * P:])
            # reconstruct lower-triangle blocks: Gs[it][:, jt] = Gs[jt][:, it].T
            for it in range(1, CT):
                for jt in range(it):
                    tps = pTri.tile([P, P], f32, name=f"tri_b{b}_{it}_{jt}",
                                    tag="tri")
                    nc.tensor.transpose(
                        out=tps,
                        in_=Gs[:, jt, it * P:(it + 1) * P],
                        identity=sb_ident,
                    )
                    nc.scalar.copy(out=Gs[:, it, jt * P:(jt + 1) * P], in_=tps)

            # row stats
            negm = pStat.tile([P, CT], f32, name=f"negm_b{b}", tag="negm")
            for it in range(CT):
                nc.vector.tensor_reduce(
                    out=negm[:, it:it + 1],
                    in_=Gs[:, it, :],
                    axis=AX.X,
                    op=Alu.max,
                    negate=True,
                )
            s_acc = pStat.tile([P, CT], f32, name=f"s_b{b}", tag="s")
            for it in range(CT):
                S = pS.tile([P, C], bf16, name=f"S_b{b}t{it}", tag="S")
                nc.scalar.activation(
                    out=S,
                    in_=Gs[:, it, :],
                    func=Exp,
                    bias=negm[:, it:it + 1],
                    scale=1.0,
                    accum_out=s_acc[:, it:it + 1],
                )
            wrec = pStat.tile([P, CT], f32, name=f"w_b{b}", tag="w")
            nc.vector.reciprocal(out=wrec, in_=s_acc)

            # transpose negm, w to row vectors; broadcast to all partitions
            vps = pPv.tile([1, C], f32, name=f"vps_b{b}", tag="vps")
            for it in range(CT):
                nc.tensor.transpose(
                    out=vps[0:1, it * P:(it + 1) * P],
                    in_=negm[:, it:it + 1],
                    identity=sb_ident,
                )
            negm_row = pRow.tile([1, C], f32, name=f"negmrow_b{b}", tag="nrow")
            nc.scalar.copy(out=negm_row, in_=vps)

            wps = pPv.tile([1, C], f32, name=f"wps_b{b}", tag="vps")
            for it in range(CT):
                nc.tensor.transpose(
                    out=wps[0:1, it * P:(it + 1) * P],
                    in_=wrec[:, it:it + 1],
                    identity=sb_ident,
                )
            w_row = pRow.tile([1, C], bf16, name=f"wrow_b{b}", tag="wrow")
            nc.scalar.copy(out=w_row, in_=wps)

            mrep_ps = pPv.tile([P, C], f32, name=f"mrepps_b{b}", tag="vps")
            nc.tensor.matmul(mrep_ps, lhsT=sb_ones_f, rhs=negm_row,
                             start=True, stop=True)
            NegM = pRow.tile([P, C], f32, name=f"negmrep_b{b}", tag="NegM")
            nc.scalar.copy(out=NegM, in_=mrep_ps)

            wrep_ps = pPv.tile([P, C], f32, name=f"wrepps_b{b}", tag="vps")
            nc.tensor.matmul(wrep_ps, lhsT=sb_ones_h, rhs=w_row,
                             start=True, stop=True)
            Wrep = pRow.tile([P, C], bf16, name=f"wrep_b{b}", tag="Wrep")
            nc.scalar.copy(out=Wrep, in_=wrep_ps)

            # T_w[j, i] = exp(G[j, i] - m_i) * w_i
            T = pTw.tile([P, CT, C], bf16, name=f"Tw_b{b}", tag="Tw")
            Tw[b] = T
            for jt in range(CT):
                tmp = pTmp.tile([P, C], f32, name=f"tmp_b{b}j{jt}", tag="tmp")
                nc.vector.tensor_tensor(
                    out=tmp, in0=Gs[:, jt, :], in1=NegM, op=Alu.add
                )
                Texp = pTmp.tile([P, C], bf16, name=f"Texp_b{b}j{jt}",
                                 tag="Texp")
                nc.scalar.activation(out=Texp, in_=tmp, func=Exp)
                nc.vector.tensor_tensor(
                    out=T[:, jt, :], in0=Texp, in1=Wrep, op=Alu.mult
                )

        def emit_transpose(b):
            # AT[b][c, jt, nt, q] = Abf[b][q, jt, nt, c]
            T = pAT.tile([P, CT, NT, P], bf16, name=f"AT_b{b}", tag="AT")
            AT[b] = T
            for jt in range(CT):
                nc.sync.dma_start_transpose(
                    out=T[:, jt, :, :],
                    in_=Abf[b][:, jt, :, :].rearrange("p k c -> p (k c)"),
                )

        def emit_mm2(b, nt0, nt1):
            A, T = Abf[b], Tw[b]
            ATb = AT[b]
            for og0 in range(nt0, nt1, OG):
                outg = pOut.tile([P, OG, C], f32, name=f"out_b{b}g{og0}",
                                 tag="out")
                for k in range(OG):
                    nt = og0 + k
                    po = pPo.tile([P, C], f32, name=f"po_b{b}n{nt}", tag="po")
                    for jt in range(CT):
                        nc.tensor.matmul(
                            po,
                            lhsT=ATb[:, jt, nt, :],
                            rhs=T[:, jt, :],
                            start=(jt == 0),
                            stop=(jt == CT - 1),
                        )
                    nc.vector.scalar_tensor_tensor(
                        out=outg[:, k, :],
                        in0=po,
                        scalar=float(gamma_val),
                        in1=A[:, :, nt, :],
                        op0=Alu.mult,
                        op1=Alu.add,
                    )
                nc.scalar.dma_start(
                    out=y[b, og0 * P:(og0 + OG) * P, :].rearrange(
                        "(k p) c -> p k c", p=P
                    ),
                    in_=outg,
                )

        # ---------------- program order (scheduling priority) ----------
        emit_loads(0)
        emit_loads(1)

        G0 = [pG.tile([P, C], f32, name=f"G_b0t{i}", tag="G") for i in range(CT)]
        emit_gram(0, 0, NT, G0)
        emit_softmax(0, G0)
        emit_transpose(0)

        G1 = [pG.tile([P, C], f32, name=f"G_b1t{i}", tag="G") for i in range(CT)]
        emit_gram(1, 0, 16, G1)
        emit_mm2(0, 0, 8)
        emit_gram(1, 16, NT, G1)
        emit_softmax(1, G1)
        emit_transpose(1)
        emit_mm2(0, 8, NT)
        emit_mm2(1, 0, NT)

    nc.compile()
    return nc


def run(inputs_arr: np.ndarray, gamma_val: float, trace: bool = False):
    """Compile + run on the 8 cores. Returns (output [16,4096,512], results)."""
    from concourse.bass_utils import run_bass_kernel_spmd

    key = round(float(gamma_val), 12)
    if key not in _BUILD_CACHE:
        _BUILD_CACHE[key] = build_bass(float(gamma_val))
    nc = _BUILD_CACHE[key]

    xs = np.ascontiguousarray(
        np.asarray(inputs_arr, dtype=np.float32).reshape(B, N, C)
    )
    eye = np.eye(P, dtype=np.float32)
    ones_f = np.ones((1, P), dtype=np.float32)
    ones_h = np.ones((1, P), dtype=np.float32).astype(_ml_bf16())
    in_maps = [
        {
            "x": xs[c * BPC:(c + 1) * BPC],
            "ident": eye,
            "ones_f": ones_f,
            "ones_h": ones_h,
        }
        for c in range(NCORES)
    ]
    res = run_bass_kernel_spmd(nc, in_maps, list(range(NCORES)), trace=trace)
    out = np.concatenate([res.results[c]["y"] for c in range(NCORES)], axis=0)
    return out.reshape(B, H, W, C), res


def kernel(inputs: np.ndarray, gamma: np.ndarray) -> np.ndarray:
    gamma_val = float(np.asarray(gamma).reshape(-1)[0])
    out, _ = run(inputs, gamma_val, trace=False)
    return out.astype(np.float32)


if __name__ == "__main__":
    rng = np.random.default_rng(0)
    inp = rng.standard_normal((B, H, W, C), dtype=np.float32)
    gam = np.zeros((1,), dtype=np.float32)
    out = kernel(inp, gam)
    print("shape", out.shape, "dtype", out.dtype)
    print("max|out - inp| =", np.abs(out - inp).max())


# revision 11
# speedup vs baseline: 1.0872x; 1.0872x over previous
"""Trainium2 Bass kernel for the CAM (channel attention module) problem.

Computation (per batch b):
    A = inputs[b] reshaped [N=4096, C=512]
    G = A^T A                       (channel Gram matrix, [C, C])
    attn = softmax(G, axis=-1)
    out[b] = gamma * (A @ attn^T) + A

Distribution: pure data-parallel over the batch dim: 16 batches over 8
NeuronCores = 2 batches/core. No collectives.

Per-core dataflow (per batch, restructured for cross-batch pipelining):
  - Input is loaded via SWDGE cast-DMA (fp32 HBM -> bf16 SBUF), in a
    jt-major layout [p, jt, nt, c].  No fp32 copy of A is kept on chip;
    the residual add reads the bf16 copy (error ~2^-9 rel, far inside
    the 2e-2 gate).  This halves SBUF footprint so both batches'
    A / A^T live simultaneously -> full cross-batch overlap.
  - Gram matmul (bf16, upper-triangle blocks only) accumulates in 4
    PSUM banks; right after the last accumulation the banks are copied
    to SBUF (Gsb) so the next batch's Gram can reuse them immediately.
  - Lower-triangle blocks reconstructed by PE transposes of Gsb.
  - Softmax: row max (DVE), S = exp(G - m) on ACT with accum_out row
    sums, 1/s via DVE reciprocal; -m and 1/s are transposed to row
    vectors (PE) and broadcast across partitions with rank-1 matmuls.
  - T_w[j, i] = exp(G[j, i] - m_i) * w_i via DVE add + ACT exp + DVE mul.
  - A^T tiles for the second matmul come from the DMA xbar transpose
    (SBUF->SBUF, bf16, contiguous source).
  - Second matmul psum = A @ T_w accumulated over 4 channel blocks.
  - Residual out = psum * gamma + A_bf16 on DVE, stored in 1MB chunks.
"""

import sys

if "/opt/trn_rl_repo" not in sys.path:
    sys.path.insert(0, "/opt/trn_rl_repo")

import numpy as np

B, H, W, C = 16, 64, 64, 512
N = H * W                 # 4096
NCORES = 8
BPC = B // NCORES         # batches per core = 2
P = 128                   # partitions
NT = N // P               # 32 n-tiles
CT = C // P               # 4 channel tiles

# load chunks (in n-tiles)
LCH = 4                   # n-tiles per load chunk
NCH = NT // LCH           # 8 chunks per batch
OG = 2                    # n-tiles per output store group

_BUILD_CACHE = {}


def _ml_bf16():
    import ml_dtypes

    return np.dtype(ml_dtypes.bfloat16)


def build_bass(gamma_val: float):
    import concourse.bass as bass
    import concourse.bacc as bacc
    import concourse.tile as tile
    from concourse import mybir
    from contextlib import ExitStack

    f32 = mybir.dt.float32
    bf16 = mybir.dt.bfloat16
    Exp = mybir.ActivationFunctionType.Exp
    Alu = mybir.AluOpType
    AX = mybir.AxisListType

    nc = bacc.Bacc("TRN2", target_bir_lowering=False)
    x = nc.dram_tensor("x", [BPC, N, C], f32, kind="ExternalInput")
    ident = nc.dram_tensor("ident", [P, P], f32, kind="ExternalInput")
    ones_f = nc.dram_tensor("ones_f", [1, P], f32, kind="ExternalInput")
    ones_h = nc.dram_tensor("ones_h", [1, P], bf16, kind="ExternalInput")
    y = nc.dram_tensor("y", [BPC, N, C], f32, kind="ExternalOutput")

    with tile.TileContext(nc) as tc, ExitStack() as ctx:
        singles = ctx.enter_context(tc.tile_pool(name="singles", bufs=1))
        pStage = ctx.enter_context(tc.tile_pool(name="pStage", bufs=2))
        pAbf = ctx.enter_context(tc.tile_pool(name="pAbf", bufs=2))
        pAT = ctx.enter_context(tc.tile_pool(name="pAT", bufs=2))
        pGsb = ctx.enter_context(tc.tile_pool(name="pGsb", bufs=2))
        pTw = ctx.enter_context(tc.tile_pool(name="pTw", bufs=2))
        pS = ctx.enter_context(tc.tile_pool(name="pS", bufs=1))
        pStat = ctx.enter_context(tc.tile_pool(name="pStat", bufs=10))
        pRow = ctx.enter_context(tc.tile_pool(name="pRow", bufs=2))
        pTmp = ctx.enter_context(tc.tile_pool(name="pTmp", bufs=2))
        pOut = ctx.enter_context(tc.tile_pool(name="pOut", bufs=2))
        pG = ctx.enter_context(tc.tile_pool(name="pG", bufs=4, space="PSUM"))
        pPo = ctx.enter_context(tc.tile_pool(name="pPo", bufs=2, space="PSUM"))
        pPv = ctx.enter_context(tc.tile_pool(name="pPv", bufs=1, space="PSUM"))
        pTri = ctx.enter_context(tc.tile_pool(name="pTri", bufs=1, space="PSUM"))

        sb_ident = singles.tile([P, P], f32)
        nc.gpsimd.dma_start(out=sb_ident, in_=ident[:, :])
        sb_ones_f = singles.tile([1, P], f32)
        nc.gpsimd.dma_start(out=sb_ones_f, in_=ones_f[:, :])
        sb_ones_h = singles.tile([1, P], bf16)
        nc.gpsimd.dma_start(out=sb_ones_h, in_=ones_h[:, :])

        # ---------------- per-batch state ----------------
        Abf = [None, None]
        AT = [None, None]
        Gsb = [None, None]
        Tw = [None, None]

        def emit_alloc_A(b):
            # Abf[b][p, jt, k, c] = bf16(x[b, k*128 + p, jt*128 + c])
            Abf[b] = pAbf.tile([P, CT, NT, P], bf16, name=f"Abf_b{b}", tag="Abf")

        def emit_load_chunk(b, ch):
            # HWDGE fp32 load (2KB contiguous rows) into rotating staging,
            # then cast to bf16 jt-major on gpsimd/vector/scalar.
            A = Abf[b]
            s = ch * LCH
            st = pStage.tile([P, LCH, C], f32, name=f"st_b{b}c{ch}", tag="st")
            src = x[b, s * P:(s + LCH) * P, :].rearrange("(k p) c -> p k c", p=P)
            nc.scalar.dma_start(out=st, in_=src)
            for jt in range(CT):
                dst = A[:, jt, s:s + LCH, :]
                csrc = st[:, :, jt * P:(jt + 1) * P]
                if jt < 2:
                    nc.gpsimd.tensor_copy(out=dst, in_=csrc)
                elif jt == 2:
                    nc.vector.tensor_copy(out=dst, in_=csrc)
                else:
                    nc.scalar.copy(out=dst, in_=csrc)

        def emit_gram(b, nt0, nt1, G):
            A = Abf[b]
            for nt in range(nt0, nt1):
                for ci in range(CT):
                    nc.tensor.matmul(
                        G[ci][:, ci * P:],
                        lhsT=A[:, ci, nt, :],
                        rhs=A[:, ci:, nt, :],
                        start=(nt == 0),
                        stop=(nt == NT - 1),
                    )

        def emit_softmax(b, G):
            # copy upper-tri G rows out of PSUM (frees banks for next batch)
            Gs = pGsb.tile([P, CT, C], f32, name=f"Gsb_b{b}", tag="Gsb")
            Gsb[b] = Gs
            for ci in range(CT):
                nc.scalar.copy(out=Gs[:, ci, ci * P:], in_=G[ci][:, ci * P:])
            # reconstruct lower-triangle blocks: Gs[it][:, jt] = Gs[jt][:, it].T
            for it in range(1, CT):
                for jt in range(it):
                    tps = pTri.tile([P, P], f32, name=f"tri_b{b}_{it}_{jt}",
                                    tag="tri")
                    nc.tensor.transpose(
                        out=tps,
                        in_=Gs[:, jt, it * P:(it + 1) * P],
                        identity=sb_ident,
                    )
                    nc.scalar.copy(out=Gs[:, it, jt * P:(jt + 1) * P], in_=tps)

            # row stats
            negm = pStat.tile([P, CT], f32, name=f"negm_b{b}", tag="negm")
            for it in range(CT):
                nc.vector.tensor_reduce(
                    out=negm[:, it:it + 1],
                    in_=Gs[:, it, :],
                    axis=AX.X,
                    op=Alu.max,
                    negate=True,
                )
            s_acc = pStat.tile([P, CT], f32, name=f"s_b{b}", tag="s")
            for it in range(CT):
                S = pS.tile([P, C], bf16, name=f"S_b{b}t{it}", tag="S")
                nc.scalar.activation(
                    out=S,
                    in_=Gs[:, it, :],
                    func=Exp,
                    bias=negm[:, it:it + 1],
                    scale=1.0,
                    accum_out=s_acc[:, it:it + 1],
                )
            wrec = pStat.tile([P, CT], f32, name=f"w_b{b}", tag="w")
            nc.vector.reciprocal(out=wrec, in_=s_acc)

            # transpose negm, w to row vectors; broadcast to all partitions
            vps = pPv.tile([1, C], f32, name=f"vps_b{b}", tag="vps")
            for it in range(CT):
                nc.tensor.transpose(
                    out=vps[0:1, it * P:(it + 1) * P],
                    in_=negm[:, it:it + 1],
                    identity=sb_ident,
                )
            negm_row = pRow.tile([1, C], f32, name=f"negmrow_b{b}", tag="nrow")
            nc.scalar.copy(out=negm_row, in_=vps)

            wps = pPv.tile([1, C], f32, name=f"wps_b{b}", tag="vps")
            for it in range(CT):
                nc.tensor.transpose(
                    out=wps[0:1, it * P:(it + 1) * P],
                    in_=wrec[:, it:it + 1],
                    identity=sb_ident,
                )
            w_row = pRow.tile([1, C], bf16, name=f"wrow_b{b}", tag="wrow")
            nc.scalar.copy(out=w_row, in_=wps)

            mrep_ps = pPv.tile([P, C], f32, name=f"mrepps_b{b}", tag="vps")
            nc.tensor.matmul(mrep_ps, lhsT=sb_ones_f, rhs=negm_row,
                             start=True, stop=True)
            NegM = pRow.tile([P, C], f32, name=f"negmrep_b{b}", tag="NegM")
            nc.scalar.copy(out=NegM, in_=mrep_ps)

            wrep_ps = pPv.tile([P, C], f32, name=f"wrepps_b{b}", tag="vps")
            nc.tensor.matmul(wrep_ps, lhsT=sb_ones_h, rhs=w_row,
                             start=True, stop=True)
            Wrep = pRow.tile([P, C], bf16, name=f"wrep_b{b}", tag="Wrep")
            nc.scalar.copy(out=Wrep, in_=wrep_ps)

            # T_w[j, i] = exp(G[j, i] - m_i) * w_i
            T = pTw.tile([P, CT, C], bf16, name=f"Tw_b{b}", tag="Tw")
            Tw[b] = T
            for jt in range(CT):
                tmp = pTmp.tile([P, C], f32, name=f"tmp_b{b}j{jt}", tag="tmp")
                nc.vector.tensor_tensor(
                    out=tmp, in0=Gs[:, jt, :], in1=NegM, op=Alu.add
                )
                Texp = pS.tile([P, C], bf16, name=f"Texp_b{b}j{jt}",
                                tag="S")
                nc.scalar.activation(out=Texp, in_=tmp, func=Exp)
                nc.vector.tensor_tensor(
                    out=T[:, jt, :], in0=Texp, in1=Wrep, op=Alu.mult
                )

        def emit_transpose(b):
            # AT[b][c, jt, nt, q] = Abf[b][q, jt, nt, c]
            T = pAT.tile([P, CT, NT, P], bf16, name=f"AT_b{b}", tag="AT")
            AT[b] = T
            for jt in range(CT):
                nc.sync.dma_start_transpose(
                    out=T[:, jt, :, :],
                    in_=Abf[b][:, jt, :, :].rearrange("p k c -> p (k c)"),
                )

        def emit_mm2(b, nt0, nt1):
            A, T = Abf[b], Tw[b]
            ATb = AT[b]
            for og0 in range(nt0, nt1, OG):
                outg = pOut.tile([P, OG, C], f32, name=f"out_b{b}g{og0}",
                                 tag="out")
                for k in range(OG):
                    nt = og0 + k
                    po = pPo.tile([P, C], f32, name=f"po_b{b}n{nt}", tag="po")
                    for jt in range(CT):
                        nc.tensor.matmul(
                            po,
                            lhsT=ATb[:, jt, nt, :],
                            rhs=T[:, jt, :],
                            start=(jt == 0),
                            stop=(jt == CT - 1),
                        )
                    nc.vector.scalar_tensor_tensor(
                        out=outg[:, k, :],
                        in0=po,
                        scalar=float(gamma_val),
                        in1=A[:, :, nt, :],
                        op0=Alu.mult,
                        op1=Alu.add,
                    )
                nc.scalar.dma_start(
                    out=y[b, og0 * P:(og0 + OG) * P, :].rearrange(
                        "(k p) c -> p k c", p=P
                    ),
                    in_=outg,
                )

        # ---------------- program order (scheduling priority) ----------
        emit_alloc_A(0)
        emit_alloc_A(1)

        G0 = [pG.tile([P, C], f32, name=f"G_b0t{i}", tag="G") for i in range(CT)]
        for ch in range(NCH):
            emit_load_chunk(0, ch)
            emit_gram(0, ch * LCH, (ch + 1) * LCH, G0)
        for ch in range(NCH):
            emit_load_chunk(1, ch)
        emit_softmax(0, G0)
        emit_transpose(0)

        G1 = [pG.tile([P, C], f32, name=f"G_b1t{i}", tag="G") for i in range(CT)]
        emit_gram(1, 0, 16, G1)
        emit_mm2(0, 0, 8)
        emit_gram(1, 16, NT, G1)
        emit_softmax(1, G1)
        emit_transpose(1)
        emit_mm2(0, 8, NT)
        emit_mm2(1, 0, NT)

    nc.compile()
    return nc


def run(inputs_arr: np.ndarray, gamma_val: float, trace: bool = False):
    """Compile + run on the 8 cores. Returns (output [16,4096,512], results)."""
    from concourse.bass_utils import run_bass_kernel_spmd

    key = round(float(gamma_val), 12)
    if key not in _BUILD_CACHE:
        _BUILD_CACHE[key] = build_bass(float(gamma_val))
    nc = _BUILD_CACHE[key]

    xs = np.ascontiguousarray(
        np.asarray(inputs_arr, dtype=np.float32).reshape(B, N, C)
    )
    eye = np.eye(P, dtype=np.float32)
    ones_f = np.ones((1, P), dtype=np.float32)
    ones_h = np.ones((1, P), dtype=np.float32).astype(_ml_bf16())
    in_maps = [
        {
            "x": xs[c * BPC:(c + 1) * BPC],
            "ident": eye,
            "ones_f": ones_f,
            "ones_h": ones_h,
        }
        for c in range(NCORES)
    ]
    res = run_bass_kernel_spmd(nc, in_maps, list(range(NCORES)), trace=trace)
    out = np.concatenate([res.results[c]["y"] for c in range(NCORES)], axis=0)
    return out.reshape(B, H, W, C), res


def kernel(inputs: np.ndarray, gamma: np.ndarray) -> np.ndarray:
    gamma_val = float(np.asarray(gamma).reshape(-1)[0])
    out, _ = run(inputs, gamma_val, trace=False)
    return out.astype(np.float32)


if __name__ == "__main__":
    rng = np.random.default_rng(0)
    inp = rng.standard_normal((B, H, W, C), dtype=np.float32)
    gam = np.zeros((1,), dtype=np.float32)
    out = kernel(inp, gam)
    print("shape", out.shape, "dtype", out.dtype)
    print("max|out - inp| =", np.abs(out - inp).max())
